# revision 1
# baseline (speedup 1.0000x reference)
"""Distributed Trainium2 kernel for a 2-relation GNN message-passing layer.

agg = x @ W_self.T + sum_r scatter_add(x[src_r] @ W_r.T, tgt_r)

Strategy (8 NeuronCores, SPMD, no collectives):
- Targets sharded: core c owns rows [c*62500, (c+1)*62500) of the output.
- x (bf16) replicated to every core as 16 bucket tensors of 31250 rows
  (dma_gather uses int16 indices, so a gather table must stay < 32768 rows).
- Phase A: per (relation, tgt-half, src-bucket), dma_gather the edges' source
  rows (bucket-local int16 idxs) and write them to a DRAM staging buffer in a
  known order.  Staging is ordered by (relation, tgt-half); each half holds
  < 32768 rows so it can itself be a gather table.
- Phase B: per (relation, 512-target window), dma_gather the window's edge rows
  from staging in target-sorted order, build a one-hot matrix T[slot, t] on
  DVE (compare streamed target values against an iota row), and matmul-
  accumulate gT_w[f, t] = sum_slots G[slot, f] * T[slot, t] in PSUM.  This is
  the scatter-add, done by TensorE -- no DMA scatter, no ordering hazards.
- Per window: out_w[t, o] = xT_w.T @ Wself^T + g0T_w.T @ W0^T + g1T_w.T @ W1^T
  accumulated in a second PSUM bank, then written to the output with one
  sequential DMA.  Self term uses a host-transposed x_own^T input.

All instruction shapes are identical across cores (SPMD); per-core variation
lives in the index tensors.  Unused trailing slots gather row 0 (a valid row: the ucode/decode descriptor
bookkeeping must agree, so every slot is a real descriptor), and their one-hot
target value is -1 (matches no column -> contributes zero).
"""
import os
import sys
import types

import numpy as np

sys.path.insert(0, "/opt/trn_rl_repo")

N = 500_000
D = 128
NUM_REL = 2
NCORE = 8
TPC = N // NCORE            # 62500 targets per core
NB = 16                     # src buckets
BROWS = N // NB             # 31250 rows per bucket
WIN = 512                   # aggregation window (one PSUM bank: 512 f32)
NWIN = (TPC + WIN - 1) // WIN          # 123 windows (last = 36 targets)
QB = [0, 15360, 30720, 46080, 62500]   # window-aligned target quarters
NQ = 4
ACAPS_Q = [(1024, 128), (1024, 128), (1024, 128), (1024, 256)]
BCAP = 640                  # phase-B slots per (rel, window) chunk
NQUEUE = 4


def _register_profile_hook():
    if "antenv.axon_hooks" in sys.modules:
        return
    mod = types.ModuleType("antenv.axon_hooks")
    state = {"h": None}
    mod.set_axon_ntff_profile_hook = lambda h: state.__setitem__("h", h)
    mod.get_axon_ntff_profile_hook = lambda: state["h"]
    sys.modules["antenv.axon_hooks"] = mod
    try:
        from trn_agent_boot.trn_boot import _ntff_profile_via_ctypes
        mod.set_axon_ntff_profile_hook(
            _ntff_profile_via_ctypes("/opt/axon/libaxon_pjrt.so"))
    except Exception:
        pass


def _achunks():
    """Static phase-A chunk table: (rel, quarter, bucket, cap, staging_row_off).
    Chunks of one (rel, quarter) occupy consecutive rows of stage[rel][q]."""
    table = []
    rows_q = [0] * NQ
    for r in range(NUM_REL):
        for q in range(NQ):
            off = 0
            for b in range(NB):
                for cap in ACAPS_Q[q]:
                    table.append((r, q, b, cap, off))
                    off += cap
            rows_q[q] = off
            assert off <= 32767, off
    return table, rows_q


ACHUNKS, STAGE_ROWS_Q = _achunks()

# windows: (w, quarter, t0, tn)
WINDOWS = []
for w in range(NWIN):
    t0 = w * WIN
    tn = min(WIN, TPC - t0)
    q = max(i for i in range(NQ) if QB[i] <= t0)
    WINDOWS.append((w, q, t0, tn))


def _pack_core(edge_indices, core):
    """Build per-core index tensors.

    Returns:
      agidx [A_SLOTS] int16   phase-A gather idxs (bucket-local src), -1 pad
      bgidx [B_SLOTS] int16   phase-B gather idxs (staging row), -1 pad
      btv   [B_SLOTS] f32     phase-B target value local to window, -1 pad
    """
    lo = core * TPC
    # pads gather row 0 (valid); their one-hot target value is -1 -> zero
    agidx = np.zeros(sum(c[3] for c in ACHUNKS), dtype=np.int16)
    bgidx = np.zeros(len(WINDOWS) * NUM_REL * BCAP, dtype=np.int16)
    btv = np.full(len(WINDOWS) * NUM_REL * BCAP, -1.0, dtype=np.float32)
    avalid = np.zeros(len(ACHUNKS), dtype=np.int64)
    bvalid = np.zeros(len(WINDOWS) * NUM_REL, dtype=np.int64)

    for r in range(NUM_REL):
        src = np.asarray(edge_indices[r, 0])
        tgt = np.asarray(edge_indices[r, 1])
        m = (tgt >= lo) & (tgt < lo + TPC)
        s = src[m]
        t = (tgt[m] - lo).astype(np.int64)
        for h in range(NQ):
            hm = (t >= QB[h]) & (t < QB[h + 1])
            sh, th = s[hm], t[hm]
            order = np.argsort(th, kind="stable")
            sh, th = sh[order], th[order]
            b = sh // BROWS
            # staging row for each edge: chunks are bucket-major
            stage_row = np.empty(sh.shape[0], dtype=np.int64)
            # phase-A fill
            for bb in range(NB):
                bm = b == bb
                es = (sh[bm] - bb * BROWS).astype(np.int16)
                chunks = [(cap, soff, fb, ci) for ci, ((rr, hh, bbx, cap, soff), fb)
                          in enumerate(zip(ACHUNKS, _AFLAT))
                          if rr == r and hh == h and bbx == bb]
                n = es.shape[0]
                total_cap = sum(c[0] for c in chunks)
                if n > total_cap:
                    raise RuntimeError(f"phase-A overflow r{r} h{h} b{bb}: {n}")
                pos = 0
                rows = np.empty(n, dtype=np.int64)
                for cap, soff, fb, ci in chunks:
                    k = min(cap, n - pos)
                    if k > 0:
                        agidx[fb:fb + k] = es[pos:pos + k]
                        rows[pos:pos + k] = soff + np.arange(k)
                        pos += k
                    avalid[ci] = k
                stage_row[np.nonzero(bm)[0]] = rows
            # phase-B fill: windows over this half
            for (w, wh, t0, tn) in WINDOWS:
                if wh != h:
                    continue
                wm = (th >= t0) & (th < t0 + tn)
                er = stage_row[wm]
                et = th[wm] - t0
                k = er.shape[0]
                if k > BCAP:
                    raise RuntimeError(f"phase-B overflow r{r} w{w}: {k}")
                boff = (w * NUM_REL + r) * BCAP
                bgidx[boff:boff + k] = er.astype(np.int16)
                btv[boff:boff + k] = et.astype(np.float32)
                bvalid[w * NUM_REL + r] = k
    return agidx, bgidx, btv, avalid, bvalid


# flat slot offsets for phase-A chunks (concatenated across (rel, half))
_AFLAT = []
_fb = 0
for (_r, _h, _b, _cap, _soff) in ACHUNKS:
    _AFLAT.append(_fb)
    _fb += _cap
A_SLOTS = _fb
B_SLOTS = len(WINDOWS) * NUM_REL * BCAP


def _wrap16(idx_flat):
    n = idx_flat.shape[0]
    a = idx_flat.reshape(n // 16, 16).T
    return np.tile(a, (8, 1)).copy()


def _slotmaj(v, width):
    """[n] -> [128, n//128 * width]-style slot-major layout [p, blk] where
    slot i -> [i%128, i//128] (gather output layout)."""
    n = v.shape[0]
    return np.ascontiguousarray(v.reshape(n // 128, 128).T)


def _build_program(aregs, bregs):
    import concourse.bacc as bacc
    import concourse.tile as tile
    from concourse import mybir

    nc = bacc.Bacc("TRN2", debug=False, num_swdge_queues=NQUEUE)
    dt = mybir.dt

    xb = [nc.dram_tensor(f"xb{k}", [BROWS, D], dt.bfloat16, kind="ExternalInput")
          for k in range(NB)]
    xto = nc.dram_tensor("xto", [D, TPC], dt.bfloat16, kind="ExternalInput")
    wt = nc.dram_tensor("wt", [D, 3 * D], dt.bfloat16, kind="ExternalInput")
    agidx_d = nc.dram_tensor("agidx", [128, A_SLOTS // 16], dt.int16, kind="ExternalInput")
    bgidx_d = nc.dram_tensor("bgidx", [128, B_SLOTS // 16], dt.int16, kind="ExternalInput")
    btv_d = nc.dram_tensor("btv", [128, B_SLOTS // 128], dt.float16, kind="ExternalInput")
    iota_d = nc.dram_tensor("iota", [128, WIN], dt.float16, kind="ExternalInput")
    stage = [[nc.dram_tensor(f"stage{r}{h}", [STAGE_ROWS_Q[h], D], dt.bfloat16,
                             kind="ExternalOutput")
              for h in range(NQ)] for r in range(NUM_REL)]
    out_d = nc.dram_tensor("out", [TPC, D], dt.float32, kind="ExternalOutput")

    with tile.TileContext(nc) as tc:
        with (
            tc.tile_pool(name="const", bufs=1) as cpool,
            tc.tile_pool(name="ag", bufs=6) as agpool,
            tc.tile_pool(name="bg", bufs=4) as bgpool,
            tc.tile_pool(name="oh", bufs=8) as ohpool,
            tc.tile_pool(name="gsb", bufs=6) as gsbpool,
            tc.tile_pool(name="xt", bufs=3) as xtpool,
            tc.tile_pool(name="psA", bufs=3, space="PSUM") as psA,
            tc.tile_pool(name="psB", bufs=4, space="PSUM") as psB,
        ):
            wt_sb = cpool.tile([D, 3 * D], dt.bfloat16)
            nc.sync.dma_start(wt_sb[:], wt[:])
            iota_sb = cpool.tile([128, WIN], dt.float16)
            nc.sync.dma_start(iota_sb[:], iota_d[:])
            agidx_sb = cpool.tile([128, A_SLOTS // 16], dt.int16)
            nc.sync.dma_start(agidx_sb[:], agidx_d[:])
            bgidx_sb = cpool.tile([128, B_SLOTS // 16], dt.int16)
            nc.sync.dma_start(bgidx_sb[:], bgidx_d[:])
            btv_sb = cpool.tile([128, B_SLOTS // 128], dt.float16)
            nc.sync.dma_start(btv_sb[:], btv_d[:])

            # ---- phase A: gather x rows into staging ----
            # Emit quarter-major with relations interleaved so each quarter's
            # staging (both rels) completes early and phase B overlaps phase A.
            _aorder = list(range(len(ACHUNKS)))
            for ci in _aorder:
                (r, h, b, cap, soff) = ACHUNKS[ci]
                fb = _AFLAT[ci]
                g = agpool.tile([128, 10, D], dt.bfloat16, tag="ag")
                nblk = cap // 128
                nc.gpsimd.dma_gather(
                    g[:, :nblk, :], xb[b][:],
                    agidx_sb[:, fb // 16:(fb + cap) // 16],
                    cap, int(aregs[ci]), D, queue_num=ci % NQUEUE,
                )
                nc.sync.dma_start(
                    stage[r][h][soff:soff + cap, :].rearrange(
                        "(j p) o -> p j o", p=128),
                    g[:, :nblk, :],
                )

            # ---- phase B: per (window, rel) gather + one-hot aggregation ----
            _maxwin = int(os.environ.get("KMAXWIN", "123"))
            _phase_a_only = os.environ.get("KPHASE", "") == "A"
            gb_bufs = []
            for i in range(8):
                t = cpool.tile([128, BCAP // 128, D], dt.bfloat16, tag=f"gbb{i}")
                nc.vector.memset(t[:], 0.0)
                gb_bufs.append(t)
            gb_rot = 0
            for (w, h, t0, tn) in WINDOWS:
                if _phase_a_only or w >= _maxwin:
                    continue
                outp = psB.tile([128, 4, D], dt.float32, tag="psB")
                # self term first
                xt_t = xtpool.tile([D, WIN], dt.bfloat16, tag="xt")
                nc.sync.dma_start(xt_t[:, :tn], xto[:, t0:t0 + tn])
                nsub_t = (tn + 127) // 128
                for j in range(nsub_t):
                    wdt = min(128, tn - j * 128)
                    nc.tensor.matmul(
                        outp[:wdt, j, :],
                        xt_t[:, j * 128:j * 128 + wdt],
                        wt_sb[:, 0:D],
                        start=(j == 0), stop=False,
                    )
                for r in range(NUM_REL):
                    boff = (w * NUM_REL + r) * BCAP
                    gb = gb_bufs[gb_rot % 8]
                    gb_rot += 1
                    nc.gpsimd.dma_gather(
                        gb[:], stage[r][h][:],
                        bgidx_sb[:, boff // 16:(boff + BCAP) // 16],
                        BCAP, int(bregs[w * NUM_REL + r]), D, transpose=False,
                        queue_num=(w * NUM_REL + r) % NQUEUE,
                    )
                    gps = psA.tile([128, WIN], dt.float32, tag="psA")
                    nblk_v = (int(bregs[w * NUM_REL + r]) + 127) // 128
                    for j in range(nblk_v):
                        oh = ohpool.tile([128, WIN], dt.bfloat16, tag="oh")
                        nc.vector.tensor_tensor(
                            out=oh[:, :tn],
                            in0=btv_sb[:, boff // 128 + j:boff // 128 + j + 1]
                                .to_broadcast([128, tn]),
                            in1=iota_sb[:, :tn],
                            op=mybir.AluOpType.is_equal,
                        )
                        nc.tensor.matmul(
                            gps[:, :tn],
                            gb[:, j, :],
                            oh[:, :tn],
                            start=(j == 0), stop=(j == nblk_v - 1),
                        )
                    gsb = gsbpool.tile([128, WIN], dt.bfloat16, tag="gsb")
                    nc.scalar.copy(out=gsb[:, :tn], in_=gps[:, :tn])
                    for j in range(nsub_t):
                        wdt = min(128, tn - j * 128)
                        nc.tensor.matmul(
                            outp[:wdt, j, :],
                            gsb[:, j * 128:j * 128 + wdt],
                            wt_sb[:, (1 + r) * D:(2 + r) * D],
                            start=False,
                            stop=(r == NUM_REL - 1 and j == nsub_t - 1),
                        )
                osb = xtpool.tile([128, 4, D], dt.float32, tag="osb")
                nc.scalar.copy(out=osb[:, :nsub_t, :], in_=outp[:, :nsub_t, :])
                if tn % 128 == 0:
                    nc.sync.dma_start(
                        out_d[t0:t0 + tn, :].rearrange("(j p) o -> p j o", p=128),
                        osb[:, :nsub_t, :],
                    )
                else:
                    nc.sync.dma_start(
                        out_d[t0:t0 + tn, :].rearrange("(j p) o -> p j o", p=tn),
                        osb[:tn, :nsub_t, :],
                    )
    nc.compile()
    return nc


_NC_CACHE = {}


def kernel(x, W0, W1, W_self, edge_indices):
    import ml_dtypes
    from concourse import bass_utils
    from concourse.bass_utils import run_bass_kernel_spmd

    _register_profile_hook()
    bass_utils.upload_artifacts = lambda tmpdir: "local://" + tmpdir

    x = np.asarray(x)
    W0 = np.asarray(W0)
    W1 = np.asarray(W1)
    W_self = np.asarray(W_self)
    edge_indices = np.asarray(edge_indices)

    bf16 = ml_dtypes.bfloat16
    x16 = x.astype(bf16)
    xbufs = [np.ascontiguousarray(x16[k * BROWS:(k + 1) * BROWS]) for k in range(NB)]
    wt = np.concatenate([W_self.T, W0.T, W1.T], axis=1).astype(bf16)
    iota = np.tile(np.arange(WIN, dtype=np.float16), (128, 1))

    packs = [_pack_core(edge_indices, c) for c in range(NCORE)]
    aregs = np.max([p[3] for p in packs], axis=0)
    bregs = np.max([p[4] for p in packs], axis=0)
    if "nc" not in _NC_CACHE:
        _NC_CACHE["nc"] = _build_program(aregs, bregs)
    nc = _NC_CACHE["nc"]

    in_maps = []
    for c in range(NCORE):
        agidx, bgidx, btv, av, bv = packs[c]
        # beyond the per-instruction register count, use -1 (ucode trims these
        # to exactly the register count on every core -> consistent bookkeeping)
        for ci, (r_, h_, b_, cap, soff) in enumerate(ACHUNKS):
            fb = _AFLAT[ci]
            agidx[fb + int(aregs[ci]):fb + cap] = -1
        for wi in range(len(WINDOWS) * NUM_REL):
            boff = wi * BCAP
            bgidx[boff + int(bregs[wi]):boff + BCAP] = -1
        im = {f"xb{k}": xbufs[k] for k in range(NB)}
        im["xto"] = np.ascontiguousarray(x16[c * TPC:(c + 1) * TPC].T)
        im["wt"] = wt
        im["agidx"] = _wrap16(agidx)
        im["bgidx"] = _wrap16(bgidx)
        im["btv"] = _slotmaj(btv.astype(np.float16), 1)
        im["iota"] = iota
        in_maps.append(im)

    trace = os.environ.get("KBENCH_TRACE", "0") == "1"
    res = run_bass_kernel_spmd(nc, in_maps, core_ids=list(range(NCORE)),
                               trace=trace)
    if trace:
        print("HW exec time:", res.exec_time_ns, "ns")
        _NC_CACHE["exec_time_ns"] = res.exec_time_ns

    out = np.empty((N, D), dtype=np.float32)
    for c in range(NCORE):
        out[c * TPC:(c + 1) * TPC] = res.results[c]["out"]
    return out



# revision 2
# speedup vs baseline: 1.2832x; 1.2832x over previous
"""Distributed Trainium2 kernel for a 2-relation GNN message-passing layer.

agg = x @ W_self.T + sum_r scatter_add(x[src_r] @ W_r.T, tgt_r)

Strategy (8 NeuronCores, SPMD, no collectives):
- Targets sharded: core c owns rows [c*62500, (c+1)*62500) of the output.
- x (bf16) replicated to every core as 16 bucket tensors of 31250 rows
  (dma_gather uses int16 indices, so a gather table must stay < 32768 rows).
- Phase A: per (quarter, relation, src-bucket), dma_gather the edges' source
  rows (bucket-local int16 idxs) and write them to a DRAM staging buffer in a
  known order.  Staging is ordered by (relation, tgt-quarter); each quarter
  holds < 32768 rows so it can itself be a gather table.  Within each chunk
  the edges are target-sorted, so phase-B windows see sorted targets.
- Phase B: per (512-target window, relation), dma_gather the window's edge
  rows from staging in target-sorted order, build narrow one-hot slices
  T[slot, t] on DVE (compare streamed target values against an iota row over
  only the block's active target range - compile-time known from the sorted
  packing, unioned across cores for SPMD) and matmul-accumulate
  gT_w[f, t] += G[slot, f] * T[slot, t] into a DVE-zeroed PSUM bank.  This is
  the scatter-add, done by TensorE with ~3x less DVE/PE work than a
  full-width one-hot.
- Per window: out_w[t, o] = xT_w.T @ Wself^T + g0T_w.T @ W0^T + g1T_w.T @ W1^T
  accumulated in a second PSUM bank, copied to bf16 and written to a
  partition-major output tensor (big DMA descriptors); the host reassembles
  and upcasts.

All instruction shapes are identical across cores (SPMD); per-core variation
lives in the index tensors.  Unused trailing slots gather row 0 (a valid row)
and their one-hot target value is -1 (matches no column -> contributes zero).
"""
import os
import sys
import types

import numpy as np

sys.path.insert(0, "/opt/trn_rl_repo")

N = 500_000
D = 128
NUM_REL = 2
NCORE = 8
TPC = N // NCORE            # 62500 targets per core
NB = 16                     # src buckets
BROWS = N // NB             # 31250 rows per bucket
WIN = 512                   # aggregation window (one PSUM bank: 512 f32)
NWIN = (TPC + WIN - 1) // WIN          # 123 windows (last = 36 targets)
QB = [0, 15360, 30720, 46080, 62500]   # window-aligned target quarters
NQ = 4
ACAPS_Q = [(1024, 128), (1024, 128), (1024, 128), (1024, 256)]
BCAP = 640                  # phase-B slots per (rel, window) chunk
NQUEUE = 4


def _register_profile_hook():
    if "antenv.axon_hooks" in sys.modules:
        return
    mod = types.ModuleType("antenv.axon_hooks")
    state = {"h": None}
    mod.set_axon_ntff_profile_hook = lambda h: state.__setitem__("h", h)
    mod.get_axon_ntff_profile_hook = lambda: state["h"]
    sys.modules["antenv.axon_hooks"] = mod
    try:
        from trn_agent_boot.trn_boot import _ntff_profile_via_ctypes
        mod.set_axon_ntff_profile_hook(
            _ntff_profile_via_ctypes("/opt/axon/libaxon_pjrt.so"))
    except Exception:
        pass


def _achunks():
    """Static phase-A chunk table: (rel, quarter, bucket, cap, staging_row_off).
    Chunks of one (rel, quarter) occupy consecutive rows of stage[rel][q]."""
    table = []
    rows_q = [0] * NQ
    for r in range(NUM_REL):
        for q in range(NQ):
            off = 0
            for b in range(NB):
                for cap in ACAPS_Q[q]:
                    table.append((r, q, b, cap, off))
                    off += cap
            rows_q[q] = off
            assert off <= 32767, off
    return table, rows_q


ACHUNKS, STAGE_ROWS_Q = _achunks()

# windows: (w, quarter, t0, tn)
WINDOWS = []
for w in range(NWIN):
    t0 = w * WIN
    tn = min(WIN, TPC - t0)
    q = max(i for i in range(NQ) if QB[i] <= t0)
    WINDOWS.append((w, q, t0, tn))


def _pack_core(edge_indices, core):
    """Build per-core index tensors.

    Returns:
      agidx [A_SLOTS] int16   phase-A gather idxs (bucket-local src), -1 pad
      bgidx [B_SLOTS] int16   phase-B gather idxs (staging row), -1 pad
      btv   [B_SLOTS] f32     phase-B target value local to window, -1 pad
    """
    lo = core * TPC
    # pads gather row 0 (valid); their one-hot target value is -1 -> zero
    agidx = np.zeros(sum(c[3] for c in ACHUNKS), dtype=np.int16)
    bgidx = np.zeros(len(WINDOWS) * NUM_REL * BCAP, dtype=np.int16)
    btv = np.full(len(WINDOWS) * NUM_REL * BCAP, -1.0, dtype=np.float32)
    avalid = np.zeros(len(ACHUNKS), dtype=np.int64)
    bvalid = np.zeros(len(WINDOWS) * NUM_REL, dtype=np.int64)

    for r in range(NUM_REL):
        src = np.asarray(edge_indices[r, 0])
        tgt = np.asarray(edge_indices[r, 1])
        m = (tgt >= lo) & (tgt < lo + TPC)
        s = src[m]
        t = (tgt[m] - lo).astype(np.int64)
        for h in range(NQ):
            hm = (t >= QB[h]) & (t < QB[h + 1])
            sh, th = s[hm], t[hm]
            order = np.argsort(th, kind="stable")
            sh, th = sh[order], th[order]
            b = sh // BROWS
            # staging row for each edge: chunks are bucket-major
            stage_row = np.empty(sh.shape[0], dtype=np.int64)
            # phase-A fill
            for bb in range(NB):
                bm = b == bb
                es = (sh[bm] - bb * BROWS).astype(np.int16)
                chunks = [(cap, soff, fb, ci) for ci, ((rr, hh, bbx, cap, soff), fb)
                          in enumerate(zip(ACHUNKS, _AFLAT))
                          if rr == r and hh == h and bbx == bb]
                n = es.shape[0]
                total_cap = sum(c[0] for c in chunks)
                if n > total_cap:
                    raise RuntimeError(f"phase-A overflow r{r} h{h} b{bb}: {n}")
                pos = 0
                rows = np.empty(n, dtype=np.int64)
                for cap, soff, fb, ci in chunks:
                    k = min(cap, n - pos)
                    if k > 0:
                        agidx[fb:fb + k] = es[pos:pos + k]
                        rows[pos:pos + k] = soff + np.arange(k)
                        pos += k
                    avalid[ci] = k
                stage_row[np.nonzero(bm)[0]] = rows
            # phase-B fill: windows over this quarter (edges stay target-sorted)
            for (w, wh, t0, tn) in WINDOWS:
                if wh != h:
                    continue
                wm = (th >= t0) & (th < t0 + tn)
                er = stage_row[wm]
                et = th[wm] - t0
                k = er.shape[0]
                if k > BCAP:
                    raise RuntimeError(f"phase-B overflow r{r} w{w}: {k}")
                boff = (w * NUM_REL + r) * BCAP
                bgidx[boff:boff + k] = er.astype(np.int16)
                btv[boff:boff + k] = et.astype(np.float32)
                bvalid[w * NUM_REL + r] = k
    return agidx, bgidx, btv, avalid, bvalid


# flat slot offsets for phase-A chunks (concatenated across (rel, quarter))
_AFLAT = []
_fb = 0
for (_r, _h, _b, _cap, _soff) in ACHUNKS:
    _AFLAT.append(_fb)
    _fb += _cap
A_SLOTS = _fb
B_SLOTS = len(WINDOWS) * NUM_REL * BCAP
OUTBLK = NWIN * 4           # 128-row col-blocks in the p-major output


def _wrap16(idx_flat):
    n = idx_flat.shape[0]
    a = idx_flat.reshape(n // 16, 16).T
    return np.tile(a, (8, 1)).copy()


def _slotmaj(v, width):
    """[n] -> [128, n//128 * width]-style slot-major layout [p, blk] where
    slot i -> [i%128, i//128] (gather output layout)."""
    n = v.shape[0]
    return np.ascontiguousarray(v.reshape(n // 128, 128).T)


def _block_ranges(packs, bregs):
    """Per (window, rel, 128-slot block): union across cores of the active
    window-local target range [lo, hi) covered by that block's edges.
    All-pad blocks get (0, 1)."""
    ranges = {}
    for (w, h, t0, tn) in WINDOWS:
        for r in range(NUM_REL):
            wi = w * NUM_REL + r
            boff = wi * BCAP
            nblk = (int(bregs[wi]) + 127) // 128
            for j in range(nblk):
                lo, hi = tn, 0
                for (agidx, bgidx, btv, av, bv) in packs:
                    blk = btv[boff + j * 128: boff + (j + 1) * 128]
                    blk = blk[blk >= 0]
                    if blk.size:
                        lo = min(lo, int(blk.min()))
                        hi = max(hi, int(blk.max()) + 1)
                if hi <= lo:
                    lo, hi = 0, 1
                ranges[(w, r, j)] = (lo, hi)
    return ranges


def _build_program(aregs, bregs, ranges):
    import concourse.bacc as bacc
    import concourse.tile as tile
    from concourse import mybir

    nc = bacc.Bacc("TRN2", debug=False, num_swdge_queues=NQUEUE)
    dt = mybir.dt

    xb = [nc.dram_tensor(f"xb{k}", [BROWS, D], dt.bfloat16, kind="ExternalInput")
          for k in range(NB)]
    xto = nc.dram_tensor("xto", [D, TPC], dt.bfloat16, kind="ExternalInput")
    wt = nc.dram_tensor("wt", [D, 3 * D], dt.bfloat16, kind="ExternalInput")
    agidx_d = nc.dram_tensor("agidx", [128, A_SLOTS // 16], dt.int16, kind="ExternalInput")
    bgidx_d = nc.dram_tensor("bgidx", [128, B_SLOTS // 16], dt.int16, kind="ExternalInput")
    btv_d = nc.dram_tensor("btv", [128, B_SLOTS // 128], dt.float16, kind="ExternalInput")
    iota_d = nc.dram_tensor("iota", [128, WIN], dt.float16, kind="ExternalInput")
    stage = [[nc.dram_tensor(f"stage{r}{h}", [STAGE_ROWS_Q[h], D], dt.bfloat16,
                             kind="ExternalOutput")
              for h in range(NQ)] for r in range(NUM_REL)]
    # partition-major output: target t = (blk*128 + p) lives at out[p, blk, :]
    out_d = nc.dram_tensor("out", [128, OUTBLK, D], dt.bfloat16,
                           kind="ExternalOutput")

    with tile.TileContext(nc) as tc:
        with (
            tc.tile_pool(name="const", bufs=1) as cpool,
            tc.tile_pool(name="ag", bufs=8) as agpool,
            tc.tile_pool(name="bg", bufs=4) as bgpool,
            tc.tile_pool(name="oh", bufs=10) as ohpool,
            tc.tile_pool(name="gsb", bufs=6) as gsbpool,
            tc.tile_pool(name="xt", bufs=4) as xtpool,
            tc.tile_pool(name="psA", bufs=4, space="PSUM") as psA,
            tc.tile_pool(name="psB", bufs=4, space="PSUM") as psB,
        ):
            wt_sb = cpool.tile([D, 3 * D], dt.bfloat16)
            nc.sync.dma_start(wt_sb[:], wt[:])
            iota_sb = cpool.tile([128, WIN], dt.float16)
            nc.sync.dma_start(iota_sb[:], iota_d[:])
            agidx_sb = cpool.tile([128, A_SLOTS // 16], dt.int16)
            nc.sync.dma_start(agidx_sb[:], agidx_d[:])
            bgidx_sb = cpool.tile([128, B_SLOTS // 16], dt.int16)
            nc.sync.dma_start(bgidx_sb[:], bgidx_d[:])
            btv_sb = cpool.tile([128, B_SLOTS // 128], dt.float16)
            nc.sync.dma_start(btv_sb[:], btv_d[:])

            # ---- phase A: gather x rows into staging ----
            # Quarter-major with relations interleaved so each quarter's
            # staging (both rels) completes early and phase B overlaps phase A.
            _aorder = sorted(range(len(ACHUNKS)),
                             key=lambda ci: (ACHUNKS[ci][1], ACHUNKS[ci][0],
                                             ACHUNKS[ci][2], -ACHUNKS[ci][3]))
            for ci in _aorder:
                (r, h, b, cap, soff) = ACHUNKS[ci]
                fb = _AFLAT[ci]
                g = agpool.tile([128, 10, D], dt.bfloat16, tag="ag")
                nblk = cap // 128
                nc.gpsimd.dma_gather(
                    g[:, :nblk, :], xb[b][:],
                    agidx_sb[:, fb // 16:(fb + cap) // 16],
                    cap, int(aregs[ci]), D, queue_num=ci % NQUEUE,
                )
                nc.sync.dma_start(
                    stage[r][h][soff:soff + cap, :].rearrange(
                        "(j p) o -> p j o", p=128),
                    g[:, :nblk, :],
                )

            # ---- phase B: per (window, rel) gather + narrow one-hot agg ----
            _maxwin = int(os.environ.get("KMAXWIN", "123"))
            _phase_a_only = os.environ.get("KPHASE", "") == "A"
            gb_bufs = []
            for i in range(10):
                t = cpool.tile([128, BCAP // 128, D], dt.bfloat16, tag=f"gbb{i}")
                nc.vector.memset(t[:], 0.0)
                gb_bufs.append(t)
            gb_rot = 0
            for (w, h, t0, tn) in WINDOWS:
                if _phase_a_only or w >= _maxwin:
                    continue
                outp = psB.tile([128, 4, D], dt.float32, tag="psB")
                # self term first
                xt_t = xtpool.tile([D, WIN], dt.bfloat16, tag="xt")
                nc.sync.dma_start(xt_t[:, :tn], xto[:, t0:t0 + tn])
                nsub_t = (tn + 127) // 128
                for j in range(nsub_t):
                    wdt = min(128, tn - j * 128)
                    nc.tensor.matmul(
                        outp[:wdt, j, :],
                        xt_t[:, j * 128:j * 128 + wdt],
                        wt_sb[:, 0:D],
                        start=(j == 0), stop=False,
                    )
                for r in range(NUM_REL):
                    boff = (w * NUM_REL + r) * BCAP
                    gb = gb_bufs[gb_rot % 10]
                    gb_rot += 1
                    nc.gpsimd.dma_gather(
                        gb[:], stage[r][h][:],
                        bgidx_sb[:, boff // 16:(boff + BCAP) // 16],
                        BCAP, int(bregs[w * NUM_REL + r]), D, transpose=False,
                        queue_num=(w * NUM_REL + r) % NQUEUE,
                    )
                    gps = psA.tile([128, WIN], dt.float32, tag="psA")
                    nc.vector.memset(gps[:, :tn], 0.0)
                    nblk_v = (int(bregs[w * NUM_REL + r]) + 127) // 128
                    for j in range(nblk_v):
                        lo, hi = ranges[(w, r, j)]
                        oh = ohpool.tile([128, WIN], dt.bfloat16, tag="oh")
                        nc.vector.tensor_tensor(
                            out=oh[:, :hi - lo],
                            in0=btv_sb[:, boff // 128 + j:boff // 128 + j + 1]
                                .to_broadcast([128, hi - lo]),
                            in1=iota_sb[:, lo:hi],
                            op=mybir.AluOpType.is_equal,
                        )
                        nc.tensor.matmul(
                            gps[:, lo:hi],
                            gb[:, j, :],
                            oh[:, :hi - lo],
                            start=False, stop=(j == nblk_v - 1),
                        )
                    gsb = gsbpool.tile([128, WIN], dt.bfloat16, tag="gsb")
                    nc.scalar.copy(out=gsb[:, :tn], in_=gps[:, :tn])
                    for j in range(nsub_t):
                        wdt = min(128, tn - j * 128)
                        nc.tensor.matmul(
                            outp[:wdt, j, :],
                            gsb[:, j * 128:j * 128 + wdt],
                            wt_sb[:, (1 + r) * D:(2 + r) * D],
                            start=False,
                            stop=(r == NUM_REL - 1 and j == nsub_t - 1),
                        )
                osb = xtpool.tile([128, 4, D], dt.bfloat16, tag="osb")
                nc.scalar.copy(out=osb[:, :nsub_t, :], in_=outp[:, :nsub_t, :])
                nc.sync.dma_start(
                    out_d[:, w * 4:w * 4 + nsub_t, :],
                    osb[:, :nsub_t, :],
                )
    nc.compile()
    return nc


_NC_CACHE = {}


def kernel(x, W0, W1, W_self, edge_indices):
    import ml_dtypes
    from concourse import bass_utils
    from concourse.bass_utils import run_bass_kernel_spmd

    _register_profile_hook()
    bass_utils.upload_artifacts = lambda tmpdir: "local://" + tmpdir

    x = np.asarray(x)
    W0 = np.asarray(W0)
    W1 = np.asarray(W1)
    W_self = np.asarray(W_self)
    edge_indices = np.asarray(edge_indices)

    bf16 = ml_dtypes.bfloat16
    x16 = x.astype(bf16)
    xbufs = [np.ascontiguousarray(x16[k * BROWS:(k + 1) * BROWS]) for k in range(NB)]
    wt = np.concatenate([W_self.T, W0.T, W1.T], axis=1).astype(bf16)
    iota = np.tile(np.arange(WIN, dtype=np.float16), (128, 1))

    packs = [_pack_core(edge_indices, c) for c in range(NCORE)]
    aregs = np.max([p[3] for p in packs], axis=0)
    bregs = np.max([p[4] for p in packs], axis=0)
    ranges = _block_ranges(packs, bregs)
    if "nc" not in _NC_CACHE:
        _NC_CACHE["nc"] = _build_program(aregs, bregs, ranges)
    nc = _NC_CACHE["nc"]

    in_maps = []
    for c in range(NCORE):
        agidx, bgidx, btv, av, bv = packs[c]
        # beyond the per-instruction register count, use -1 (ucode trims these
        # to exactly the register count on every core -> consistent bookkeeping)
        for ci, (r_, h_, b_, cap, soff) in enumerate(ACHUNKS):
            fb = _AFLAT[ci]
            agidx[fb + int(aregs[ci]):fb + cap] = -1
        for wi in range(len(WINDOWS) * NUM_REL):
            boff = wi * BCAP
            bgidx[boff + int(bregs[wi]):boff + BCAP] = -1
        im = {f"xb{k}": xbufs[k] for k in range(NB)}
        im["xto"] = np.ascontiguousarray(x16[c * TPC:(c + 1) * TPC].T)
        im["wt"] = wt
        im["agidx"] = _wrap16(agidx)
        im["bgidx"] = _wrap16(bgidx)
        im["btv"] = _slotmaj(btv.astype(np.float16), 1)
        im["iota"] = iota
        in_maps.append(im)

    trace = os.environ.get("KBENCH_TRACE", "0") == "1"
    res = run_bass_kernel_spmd(nc, in_maps, core_ids=list(range(NCORE)),
                               trace=trace)
    if trace:
        print("HW exec time:", res.exec_time_ns, "ns")
        _NC_CACHE["exec_time_ns"] = res.exec_time_ns

    out = np.empty((N, D), dtype=np.float32)
    for c in range(NCORE):
        o = np.asarray(res.results[c]["out"])          # [128, OUTBLK, 128] bf16
        o = o.transpose(1, 0, 2).reshape(OUTBLK * 128, D)[:TPC]
        out[c * TPC:(c + 1) * TPC] = o.astype(np.float32)
    return out
